# revision 7
# baseline (speedup 1.0000x reference)
"""Trainium2 Bass kernel for nn_Net_16776142258913 (binarized dense MLP).

Mathematical reduction: in eval mode, ss_bn -> clip -> ste_sign is exactly
sign-preserving (l1,l2 > 0, sigmoid mix > 0, sqrt(var)+eps > 0), so the whole
network collapses to a chain of sign matmuls:

    out = sign(sign(sign(x @ sign(W1).T) @ sign(W2).T) @ sign(W3).T) @ sign(W4).T

fc2/fc3/fc4 operate on {-1,0,+1} x {-1,+1} values: products and partial sums
are small integers, exact in fp32 PSUM, so they are computed in fp8 with
DoubleRow at full throughput, bit-exact.

fc1 (x not binarized) must get every sign(h1) right: x*2^8 is decomposed into
three integer-valued fp16 digit matrices (12/12/11 bits, scales 2^8, 2^20,
2^31).  Integer x {+-1} products and integer partial sums are exact in PSUM,
so S1 = q1@W1s is exact and B2 = q2@W1s + (q3*2^-11)@W1s is exact to ~2^-25.
h1*2^8 = S1 + B2*2^-12 combined on DVE: IEEE adds are correctly rounded, so
the computed sign equals the exact sign (residual < 784 * 2^-31).

Sharding: data-parallel over batch, 8 cores x 1024 rows; weights replicated.
No cross-core communication is needed (BN statistics cancel out of the sign
chain entirely).
"""

import os
import sys
import types

import numpy as np

import concourse.bass as bass
import concourse.mybir as mybir
from concourse import bacc, tile
from concourse.bass_utils import run_bass_kernel_spmd


def _ensure_axon_hooks():
    """bass_utils imports antenv.axon_hooks when trace=True under axon; the
    trimmed image lacks that module. Provide it (and register the real ctypes
    NTFF hook when available) so tracing works instead of crashing."""
    try:
        import antenv.axon_hooks  # noqa: F401
        return
    except ImportError:
        pass
    import antenv
    mod = types.ModuleType("antenv.axon_hooks")
    mod._hook = None
    mod.set_axon_ntff_profile_hook = lambda h: setattr(mod, "_hook", h)
    mod.get_axon_ntff_profile_hook = lambda: mod._hook
    sys.modules["antenv.axon_hooks"] = mod
    antenv.axon_hooks = mod
    try:
        from trn_agent_boot.trn_boot import _ntff_profile_via_ctypes
        mod._hook = _ntff_profile_via_ctypes("/opt/axon/libaxon_pjrt.so")
    except Exception:
        pass


_ensure_axon_hooks()

F32 = mybir.dt.float32
BF16 = mybir.dt.bfloat16
FP16 = mybir.dt.float16
FP8 = mybir.dt.float8e4

P = 128
B = 1024                 # per-core batch
KX = 784                 # fc1 contraction
KXP = 896                # padded to 7*128
NK1 = KXP // P           # 7
H = 3072
NH = H // P              # 24
NB = B // P              # 8 batch slabs per core
C_RND = float(1.5 * 2 ** 23)
N_CORES = 8

AF = mybir.ActivationFunctionType
ALU = mybir.AluOpType
DR = mybir.MatmulPerfMode.DoubleRow


def _quantize_x(nc, pq, x_dram, xT):
    """Decompose x*2^8 into integer fp16 digits, transposed into xT.

    xT[d] : [P, NB, NK1, P] fp16; xT[d][p, b, kt, n] = digit_d of batch row
    b*128+n at k = kt*128+p (zero for k >= 784).
    x = q1*2^-8 + q2*2^-20 + (q3s*2^11)*2^-31 with q3s = q3*2^-11.
    """
    for b in range(NB):
        xf = pq.tile([P, KX], F32, tag="xf", name=f"xf{b}")
        nc.sync.dma_start(out=xf, in_=x_dram[b * P:(b + 1) * P, :])

        digs = [pq.tile([P, KXP], FP16, tag=f"dig{d}", name=f"dig{d}_{b}")
                for d in range(3)]
        for d in range(3):
            nc.vector.memset(digs[d][:, KX:], 0.0)

        ta = pq.tile([P, KX], F32, tag="ta", name=f"ta{b}")
        tb = pq.tile([P, KX], F32, tag="tb", name=f"tb{b}")
        tc_ = pq.tile([P, KX], F32, tag="tc", name=f"tc{b}")

        # y1 = x*256 ; q1 = round(y1)
        nc.vector.tensor_scalar_mul(ta, xf, 256.0)                      # a = y1
        nc.scalar.activation(tb, xf, AF.Copy, bias=C_RND, scale=256.0)  # b = y1+C
        nc.vector.tensor_scalar_sub(digs[0][:, :KX], tb, C_RND)        # q1
        nc.vector.tensor_tensor(out=ta, in0=ta, in1=digs[0][:, :KX],
                                op=ALU.subtract)                        # a = r1
        # y2 = r1*4096 ; q2 = round(y2)
        nc.vector.tensor_scalar_mul(tc_, ta, 4096.0)                    # c = y2
        nc.scalar.activation(tb, ta, AF.Copy, bias=C_RND, scale=4096.0)
        nc.vector.tensor_scalar_sub(digs[1][:, :KX], tb, C_RND)        # q2
        nc.vector.tensor_tensor(out=ta, in0=tc_, in1=digs[1][:, :KX],
                                op=ALU.subtract)                        # a = r2
        # y3 = r2*2048 ; q3s = round(y3)*2^-11
        nc.scalar.activation(tb, ta, AF.Copy, bias=C_RND, scale=2048.0)
        nc.vector.tensor_scalar(out=digs[2][:, :KX], in0=tb,
                                scalar1=C_RND, scalar2=float(2.0 ** -11),
                                op0=ALU.subtract, op1=ALU.mult)         # q3s

        for d in range(3):
            nc.sync.dma_start_transpose(out=xT[d][:, b], in_=digs[d])


def _fc1(nc, pools, x_dram, w1_dram, h1T):
    """h1T[:, s, n] = sign(h1) of batch-col n, feature s*128+p, fp8."""
    pq, pw1, pcmb, ps = pools["pq"], pools["pw1"], pools["pcmb"], pools["ps"]
    xT = [pools["res"].tile([P, NB, NK1, P], FP16, tag=f"xT{d}", name=f"xT{d}")
          for d in range(3)]
    _quantize_x(nc, pq, x_dram, xT)

    for s in range(NH):
        # prep W1 slab s -> lhsT [P, NK1, P] fp16 signs (padded K zeros)
        wf = pw1.tile([P, KX], F32, tag="w1f", name=f"w1f{s}")
        nc.sync.dma_start(out=wf, in_=w1_dram[s * P:(s + 1) * P, :])
        wsgn = pw1.tile([P, KXP], FP16, tag="w1sgn", name=f"w1sgn{s}")
        nc.vector.memset(wsgn[:, KX:], 0.0)
        nc.scalar.sign(out=wsgn[:, :KX], in_=wf)
        w1t = pw1.tile([P, NK1, P], FP16, tag="w1t", name=f"w1t{s}")
        nc.sync.dma_start_transpose(out=w1t, in_=wsgn)

        for h in range(2):
            psA = ps.tile([P, 512], F32, tag="psA", name=f"psA{s}_{h}")
            psB = ps.tile([P, 512], F32, tag="psB", name=f"psB{s}_{h}")
            bsl = slice(4 * h, 4 * h + 4)
            for kt in range(NK1):
                first, last = kt == 0, kt == NK1 - 1
                nc.tensor.matmul(psA, lhsT=w1t[:, kt, :],
                                 rhs=xT[0][:, bsl, kt, :],
                                 start=first, stop=last)
                nc.tensor.matmul(psB, lhsT=w1t[:, kt, :],
                                 rhs=xT[1][:, bsl, kt, :],
                                 start=first, stop=False)
                nc.tensor.matmul(psB, lhsT=w1t[:, kt, :],
                                 rhs=xT[2][:, bsl, kt, :],
                                 start=False, stop=last)
            # h1*2^8 = S1 + B2*2^-12 ; sign -> fp8
            cmb = pcmb.tile([P, 512], F32, tag="cmb", name=f"cmb{s}_{h}")
            nc.vector.tensor_scalar_mul(cmb, psB, float(2.0 ** -12))
            nc.vector.tensor_tensor(out=cmb, in0=psA, in1=cmb, op=ALU.add)
            nc.scalar.sign(out=h1T[:, s, 512 * h:512 * (h + 1)], in_=cmb)


def _binlayer(nc, pools, w_dram, hin_T, hout_T, lname):
    """hout_T = sign(signed-matmul of hin_T with streamed weight), fp8 DR."""
    pw2, ps = pools["pw2"], pools["ps"]
    HH = H // 2
    for s in range(NH):
        wsgn = pw2.tile([P, H], BF16, tag="w2sgn", name=f"{lname}_wsgn{s}")
        for half in range(2):
            wf = pw2.tile([P, HH], F32, tag="w2f", name=f"{lname}_wf{s}_{half}")
            nc.sync.dma_start(
                out=wf, in_=w_dram[s * P:(s + 1) * P, half * HH:(half + 1) * HH])
            nc.scalar.sign(out=wsgn[:, half * HH:(half + 1) * HH], in_=wf)
        wt16 = pw2.tile([P, NH, P], BF16, tag="w2t16", name=f"{lname}_wt16_{s}")
        nc.sync.dma_start_transpose(out=wt16, in_=wsgn)
        wt8 = pw2.tile([P, NH, P], FP8, tag="w2t8", name=f"{lname}_wt8_{s}")
        nc.gpsimd.dma_start(out=wt8, in_=wt16)   # SWDGE cast bf16->fp8

        for h in range(2):
            psC = ps.tile([P, 512], F32, tag="psC", name=f"{lname}_psC{s}_{h}")
            bs = slice(512 * h, 512 * (h + 1))
            for t in range(NH // 2):
                nc.tensor.matmul(psC, lhsT=wt8[:, 2 * t:2 * t + 2, :],
                                 rhs=hin_T[:, 2 * t:2 * t + 2, bs],
                                 start=(t == 0), stop=(t == NH // 2 - 1),
                                 perf_mode=DR)
            nc.scalar.sign(out=hout_T[:, s, bs], in_=psC)


def _fc4(nc, pools, w4_dram, h3T, out_dram):
    p4, ps = pools["p4"], pools["ps"]
    # W4T: [P, NH, 16] fp8 signs, cols 10:16 zero (DR needs free stride %16)
    w4raw = p4.tile([P, NH, 10], F32, tag="w4raw", name="w4raw")
    for o in range(10):
        nc.sync.dma_start(out=w4raw[:, :, o],
                          in_=w4_dram[o].rearrange("(s p) -> p s", p=P))
    w4t8 = p4.tile([P, NH, 16], FP8, tag="w4t8", name="w4t8")
    nc.vector.memset(w4t8, 0.0)
    nc.scalar.sign(out=w4t8[:, :, :10], in_=w4raw)

    outT = p4.tile([16, B], F32, tag="outT", name="outT")
    for h in range(2):
        psD = ps.tile([16, 512], F32, tag="psD", name=f"psD{h}")
        bs = slice(512 * h, 512 * (h + 1))
        for t in range(NH // 2):
            nc.tensor.matmul(psD, lhsT=w4t8[:, 2 * t:2 * t + 2, :],
                             rhs=h3T[:, 2 * t:2 * t + 2, bs],
                             start=(t == 0), stop=(t == NH // 2 - 1),
                             perf_mode=DR)
        nc.vector.tensor_copy(out=outT[:, bs], in_=psD)
    nc.sync.dma_start(out=out_dram.rearrange("b o -> o b"), in_=outT[:10, :])


def build_nc():
    nc = bacc.Bacc("TRN2", target_bir_lowering=False, debug=False)
    x = nc.dram_tensor("x", [B, KX], F32, kind="ExternalInput")
    w1 = nc.dram_tensor("W1", [H, KX], F32, kind="ExternalInput")
    w2 = nc.dram_tensor("W2", [H, H], F32, kind="ExternalInput")
    w3 = nc.dram_tensor("W3", [H, H], F32, kind="ExternalInput")
    w4 = nc.dram_tensor("W4", [10, H], F32, kind="ExternalInput")
    out = nc.dram_tensor("out", [B, 10], F32, kind="ExternalOutput")

    with tile.TileContext(nc) as tc:
        with tc.tile_pool(name="res", bufs=1) as res, \
             tc.tile_pool(name="hbuf", bufs=1) as hbuf, \
             tc.tile_pool(name="pq", bufs=2) as pq, \
             tc.tile_pool(name="pw1", bufs=2) as pw1, \
             tc.tile_pool(name="pw2", bufs=2) as pw2, \
             tc.tile_pool(name="pcmb", bufs=2) as pcmb, \
             tc.tile_pool(name="p4", bufs=1) as p4, \
             tc.tile_pool(name="ps", bufs=2, space="PSUM") as ps:
            pools = {"res": res, "pq": pq, "pw1": pw1, "pw2": pw2,
                     "pcmb": pcmb, "p4": p4, "ps": ps}
            h1T = hbuf.tile([P, NH, B], FP8, tag="h1T", name="h1T")
            h2T = hbuf.tile([P, NH, B], FP8, tag="h2T", name="h2T")
            h3T = h1T  # h1T is dead once fc3 starts; reuse its buffer
            _fc1(nc, pools, x, w1, h1T)
            _binlayer(nc, pools, w2, h1T, h2T, "fc2")
            _binlayer(nc, pools, w3, h2T, h3T, "fc3")
            _fc4(nc, pools, w4, h3T, out)

    nc.finalize()
    return nc


_NC_CACHE = None


def kernel(**inputs) -> np.ndarray:
    global _NC_CACHE
    if _NC_CACHE is None:
        _NC_CACHE = build_nc()
    nc = _NC_CACHE

    x = np.ascontiguousarray(np.asarray(inputs["x"], dtype=np.float32))
    x = x.reshape(-1, KX)
    ws = {k: np.ascontiguousarray(np.asarray(inputs[k], dtype=np.float32))
          for k in ("W1", "W2", "W3", "W4")}

    in_maps = []
    for c in range(N_CORES):
        m = {"x": np.ascontiguousarray(x[c * B:(c + 1) * B])}
        m.update(ws)
        in_maps.append(m)

    res = run_bass_kernel_spmd(
        nc, in_maps, core_ids=list(range(N_CORES)),
        trace=bool(int(os.environ.get("KERNEL_TRACE", "0"))),
    )
    out = np.concatenate([res.results[c]["out"] for c in range(N_CORES)], axis=0)
    if getattr(res, "exec_time_ns", None) is not None:
        kernel.last_exec_time_ns = res.exec_time_ns
    kernel.last_results = res
    return out.astype(np.float32)


kernel.last_exec_time_ns = None
kernel.last_results = None
